# revision 43
# baseline (speedup 1.0000x reference)
"""Trainium2 Bass kernel for the DependencyAnalyzer GNN problem.

Computation (reference semantics):
    h = relu(features @ W_node + b_node)                  # [N, H]
    2x: agg = scatter_add(h[src] -> dst);  h = relu((h + agg) @ W_conv + b_conv)
    out = stack([ (m*h) @ (m*h).T,  h @ h.T ])            # m = (nodes == 2)

Strategy (8 NeuronCores, SPMD):
  - Host reformats the edge list into per-core dense adjacency blocks
    A'^T [src=8192, dst_local=1024] in fp8 (counts are exact), with the
    identity folded in (A' = A + I_c) so that A' @ h == h_block + agg.
    The src k-tiles are PERMUTED per core: own block first, then peers
    in ring order (c+1, ..., c+7), with features permuted identically,
    so round 2 starts on locally-available own tiles before AG1 lands
    and consumes each peer's tiles in gather-arrival order -- while the
    instruction stream stays core-uniform (peer addressing goes through
    DynSlice registers loaded from a per-core index input).
  - h is fp16 end-to-end (validated: 3.6e-3 max rel err vs the 2e-2
    gate).  Each round ends in TWO AllGather halves; measured mesh time
    is bytes-dominated (~70 GB/s/core + ~5us fixed), and the collective
    engine has a ~55-70us cold-init wall after kernel launch, so the
    split halves pipeline compute into the second mesh: round 2 runs
    during AG1b, the first sim cells during AG2a/b.
  - Both outputs are symmetric and function_deps = mask.outer * sim, so
    the device computes ONLY the upper triangle of sim: a uniform
    18-cell-per-core cover of the 136 upper [512x512] cells.  Cells run
    as tile_position row-group pairs (~2x over serial K=64 matmuls).
  - sim cells are written as bf16; the host casts, mirrors, and applies
    the fdeps mask during output assembly.
"""

import numpy as np
import ml_dtypes

import concourse.bass as bass
import concourse.mybir as mybir
import concourse.tile as tile
from concourse import masks
from concourse.bass import DynSlice
from concourse.bass_utils import run_bass_kernel_spmd

N = 8192          # nodes
NB = 1024         # nodes per core block
NCORES = 8
F = 10            # feature dim
FA = F + 1        # +1 ones row (bias fold)
H = 64            # hidden dim
KT = N // 128     # 64 src k-tiles
MT = NB // 128    # 8 own m-tiles
F32 = mybir.dt.float32
F16 = mybir.dt.float16
BF16 = mybir.dt.bfloat16
F8 = mybir.dt.float8e4
I32 = mybir.dt.int32
RELU = mybir.ActivationFunctionType.Relu

# ---- the 18-cell symmetric cover -----------------------------------------
# cell = (sigma, rho): sim[own strip sigma (512 rows)] x [rot strip rho],
# rot strip rho = absolute strip (2c + rho) % 16 (pure rotation).  rho 0,1
# are the core's own strips.  Cell (1, 8) is dropped everywhere: its pair
# {2c+1, 2c+8} is exactly core (c+4)'s (0, 9) pair, so the 19-cell
# rotation cover is uniformly redundant there.  Cells run as tile_position
# row-group pairs (rho@rows0:64, rho'@64:128).  Gathered strip rho sits at
# partition base 64*((rho//2)%2), column slot (rho//2)-1 for evens /
# 6+rho//2 for odds of the rhs tile.  Schedule per sigma: "own" runs
# before the final AllGathers, "even" after AG2a, "odd" after AG2b.
SCHED = {
    0: {"own": [(0, 1)], "even": [(4, 2), (8, 6)], "odd": [(9, 11), (13, 15)]},
    1: {"own": [(None, 1)], "even": [(12, 10), (None, 14)], "odd": [(5, 3), (9, 7)]},
}
# output column slot (x512) in out_ext for each (sigma, rho) cell
OUT_SLOT = {
    (0, 0): 0, (0, 1): 1, (0, 4): 2, (0, 2): 3, (0, 8): 4, (0, 6): 5,
    (0, 9): 6, (0, 11): 7, (0, 13): 8, (0, 15): 9,
    (1, 1): 0, (1, 12): 1, (1, 10): 2, (1, 14): 3,
    (1, 5): 4, (1, 3): 5, (1, 9): 6, (1, 7): 7,
}
# first slot and slot count of each (sigma, phase) output store
PHASE_SLOTS = {
    (0, "own"): (0, 2), (0, "even"): (2, 4), (0, "odd"): (6, 4),
    (1, "own"): (0, 1), (1, "even"): (1, 3), (1, "odd"): (4, 4),
}
# rotated-strip gather issue order = first-needed order in the tau loop
EVEN_RHO_ORDER = [4, 2, 8, 6, 12, 10, 14]
ODD_RHO_ORDER = [9, 11, 13, 15, 5, 3, 7]


def rot_table(c):
    """Absolute 512-strip index for each rotated slot rho of core c."""
    return [(2 * c + r) % 16 for r in range(16)]


def k_perm(c):
    """Per-core src k-tile permutation: perm[slot] = absolute k-tile.
    Own block (8 tiles) first, then peer (c+j)'s first-half tiles for
    j=1..7 (delivered by AG1a), then the peers' second halves (AG1b)."""
    perm = [8 * c + t for t in range(8)]
    perm += [8 * ((c + j) % 8) + t for j in range(1, 8) for t in range(4)]
    perm += [8 * ((c + j) % 8) + 4 + t for j in range(1, 8) for t in range(4)]
    return perm


LAST_RESULT = None  # BassKernelResults of the most recent run (for test harness)


def _ensure_trace_hook():
    """Best-effort: register the NTFF profiling hook for trace=True runs."""
    import sys as _sys
    import types as _types

    try:
        if "antenv.axon_hooks" in _sys.modules:
            return
        import antenv as _antenv

        mod = _types.ModuleType("antenv.axon_hooks")
        _state = {"hook": None}
        mod.set_axon_ntff_profile_hook = lambda h: _state.__setitem__("hook", h)
        mod.get_axon_ntff_profile_hook = lambda: _state["hook"]
        _sys.modules["antenv.axon_hooks"] = mod
        _antenv.axon_hooks = mod

        from trn_agent_boot.trn_boot import _ntff_profile_via_ctypes

        so_path = "/opt/axon/libaxon_pjrt.so"
        import os as _os

        if _os.path.exists(so_path):
            hook = _ntff_profile_via_ctypes(so_path)
            if hook is not None:
                mod.set_axon_ntff_profile_hook(hook)
    except Exception:
        pass


def _legalize_waits(nc, max_waits=1):
    """This walrus build accepts at most one sync-wait per lowered HW
    instruction; hoist extra waits onto standalone EventSemaphore
    instructions on the same (in-order) engine queue."""
    n_fixed = 0
    for f in nc.m.functions:
        for bb in f.blocks:
            new_list = []
            for ins in bb.instructions:
                si = ins.sync_info
                if si is not None and len(si.on_wait) > max_waits:
                    waits = list(si.on_wait)
                    for w in waits[: len(waits) - max_waits]:
                        ev = mybir.InstEventSemaphore(
                            name=f"{ins.name}-w-{w.ant_name}",
                            ins=[],
                            outs=[],
                            sync_info=mybir.SyncInfo(on_wait=[w], on_update=[]),
                            engine=ins.engine,
                        )
                        new_list.append(ev)
                    ins.sync_info = mybir.SyncInfo(
                        on_wait=waits[len(waits) - max_waits :],
                        on_update=list(si.on_update),
                    )
                    n_fixed += 1
                new_list.append(ins)
            bb.instructions = new_list
    return n_fixed


def _build_nc():
    nc = bass.Bass(num_devices=NCORES)

    # ---- external I/O (same program on all cores; per-core data differs) ----
    # features^T, k-slots permuted per core (see k_perm), split by slot
    # parity: even slots at SBUF partitions 0:33, odd at 64:97 for
    # concurrent 2-row-tile phase-1 matmul pairs.
    featEv = nc.declare_dram_parameter("featEv", [3 * FA, N // 2], BF16, isOutput=False)
    featOd = nc.declare_dram_parameter("featOd", [3 * FA, N // 2], BF16, isOutput=False)
    WnA = nc.declare_dram_parameter("W3", [3 * FA, H], BF16, isOutput=False)
    Wc16 = nc.declare_dram_parameter("Wc16", [H, H], F16, isOutput=False)
    bc = nc.declare_dram_parameter("bc", [H, 1], F32, isOutput=False)
    rot_idx = nc.declare_dram_parameter("rot_idx", [1, 7], I32, isOutput=False)
    # A'^T p-major, k-slots permuted: A_p[p, s*1024 + n] = A'^T[perm[s]*128+p, n]
    A_p = nc.declare_dram_parameter("A_p", [128, KT * NB], F8, isOutput=False)
    # out[tau*128+p, slot*512 + f]: sim cell values (see OUT_SLOT)
    out_ext = nc.declare_dram_parameter("out", [NB, 10 * 512], BF16, isOutput=True)

    # ---- internal DRAM (collective bounce buffers) ----
    ag1a_in = nc.dram_tensor("ag1a_in", [NB // 2, H], F16)
    ag1a_out = nc.dram_tensor("ag1a_out", [N // 2, H], F16, addr_space="Shared")
    ag1b_in = nc.dram_tensor("ag1b_in", [NB // 2, H], F16)
    ag1b_out = nc.dram_tensor("ag1b_out", [N // 2, H], F16, addr_space="Shared")
    # final h, fp16, T layout: AG2a carries every core's even strip (local
    # cols 0:512), AG2b the odd strip; out row r*64+k = rank r's row k
    ag2a_in = nc.dram_tensor("ag2a_in", [H, 512], F16)
    ag2a_out = nc.dram_tensor("ag2a_out", [8 * H, 512], F16, addr_space="Shared")
    ag2b_in = nc.dram_tensor("ag2b_in", [H, 512], F16)
    ag2b_out = nc.dram_tensor("ag2b_out", [8 * H, 512], F16, addr_space="Shared")
    rg = [list(range(NCORES))]

    with tile.TileContext(nc, num_cores=NCORES) as tc:
        with tc.tile_pool(name="persist", bufs=1) as persist:
            # ---------------- constants / small inputs ----------------------
            # sync queue: W then features (phase-1 critical path) then half
            # the A tiles; scalar queue: small constants + other A half.
            wn_s = persist.tile([64 + 3 * FA, H], BF16)
            nc.sync.dma_start(out=wn_s[0 : 3 * FA, :], in_=WnA[:])
            nc.sync.dma_start(out=wn_s[64 : 64 + 3 * FA, :], in_=WnA[:])
            # W_conv on both partition halves so the two dst-half W matmuls
            # can run as a tile_position row-group pair
            wc_s = persist.tile([128, H], F16)
            nc.scalar.dma_start(out=wc_s[0:H, :], in_=Wc16[:])
            nc.scalar.dma_start(out=wc_s[H:128, :], in_=Wc16[:])
            bc_s = persist.tile([H, 1], F32)
            nc.scalar.dma_start(out=bc_s[:], in_=bc[:])
            rot_s = persist.tile([1, 7], I32)
            nc.scalar.dma_start(out=rot_s[:], in_=rot_idx[:])
            ident = persist.tile([H, H], F16)
            masks.make_identity(nc, ident[:])
            dummy_s = persist.tile([1, 512], BF16)
            nc.vector.memset(dummy_s[:], 0.0)

            # ring indices (c+j)%8, j=1..7 -> registers for the per-core
            # peer reads out of the final AllGathers
            rot_vals = [
                nc.values_load(
                    rot_s[0:1, i : i + 1],
                    min_val=0,
                    max_val=7,
                    skip_runtime_bounds_check=True,
                )
                for i in range(7)
            ]
            # scratch for the register-load warm-up (see phase 1)
            rwarm = persist.tile([1, 16], F8)

            def absorb(pt, parts, free):
                # Dummy full-tile matmul: soaks up PSUM pool-boundary WAR
                # waits on PE so real matmuls stay within the ISA's sync
                # wait budget.
                nc.tensor.matmul(
                    pt[:, :],
                    dummy_s[0:1, 0:parts],
                    dummy_s[0:1, 0:free],
                    start=True,
                    stop=True,
                )

            # final h (own block, T layout, fp16), duplicated on partitions
            # 64:128 for tile_position-paired K=64 matmuls in phase 3
            hT16d = persist.tile([128, NB], F16)

            with (
                tc.tile_pool(name="apool", bufs=16) as apool,
                tc.tile_pool(name="hpool", bufs=KT) as hpool,
            ):
                # ------------- phase 1: h0 for all nodes (replicated) -------
                # Concurrent row-tile pairs: even k-slot at partitions 0:33
                # (tile (0,0)), odd at 64:97 (tile (64,0)).  PSUM is
                # evacuated on the vector engine: the scalar sequencer is
                # busy generating A-tile DMA descriptors at this point.
                h0_tiles = [None] * KT
                with (
                    tc.tile_pool(name="ph1", bufs=2) as ph1,
                    tc.tile_pool(name="pp1", bufs=4, space="PSUM") as pp1,
                ):
                    ft_halves = []
                    for half in range(2):
                        ft_h = ph1.tile(
                            [64 + 3 * FA, N // 4], BF16, tag=f"ft{half}", bufs=1
                        )
                        nc.sync.dma_start(
                            out=ft_h[0 : 3 * FA, :],
                            in_=featEv[:, half * (N // 4) : (half + 1) * (N // 4)],
                        )
                        nc.sync.dma_start(
                            out=ft_h[64 : 64 + 3 * FA, :],
                            in_=featOd[:, half * (N // 4) : (half + 1) * (N // 4)],
                        )
                        ft_halves.append(ft_h)

                    # adjacency, fp8, resident in SBUF for both rounds;
                    # alternate queues so descriptor gen is 2-wide
                    a_tiles = []
                    for j in range(16):
                        at = apool.tile([128, 4 * NB], F8, name=f"a{j}", tag="A")
                        eng = nc.sync if j % 2 == 0 else nc.scalar
                        eng.dma_start(
                            out=at[:], in_=A_p[:, j * 4 * NB : (j + 1) * 4 * NB]
                        )
                        a_tiles.append(at)

                    def a_slice(k, nh):
                        t = a_tiles[k // 4]
                        off = (k % 4) * NB + nh * 512
                        return t[:, off : off + 512]

                    # The per-queue DynSlice register loads otherwise
                    # materialize lazily at first use (~0.35us each, right
                    # in the AG2->cells gap).  Touch each register now on
                    # the queues that will run the phase-3 strip gathers
                    # (sync: even strips, scalar: odd strips) -- 1-byte
                    # dynamic reads of the read-only adjacency input,
                    # emitted behind the A-tile descriptor gens so the A
                    # load isn't delayed.
                    for i, v in enumerate(rot_vals):
                        nc.scalar.dma_start(
                            out=rwarm[0:1, i : i + 1], in_=A_p[0:1, DynSlice(v, 1)]
                        )
                        nc.sync.dma_start(
                            out=rwarm[0:1, i + 8 : i + 9],
                            in_=A_p[0:1, DynSlice(v, 1)],
                        )

                    first_p1 = True
                    for half in range(2):
                        ft_h = ft_halves[half]
                        for j in range(KT // 4):  # 16 pairs per half
                            csl = slice(j * 128, (j + 1) * 128)
                            for par, pbase in ((0, 0), (1, 64)):
                                k = half * (KT // 2) + 2 * j + par
                                ps = pp1.tile([128, H], F32, tag="p64", bufs=4)
                                if first_p1:
                                    absorb(ps, 128, H)
                                    first_p1 = False
                                nc.tensor.matmul(
                                    ps[:],
                                    ft_h[pbase : pbase + 3 * FA, csl],
                                    wn_s[pbase : pbase + 3 * FA, :],
                                    start=True,
                                    stop=True,
                                    tile_position=(pbase, 0),
                                    skip_group_check=True,
                                )
                                hl = hpool.tile([128, H], F16, name=f"h0_{k}", tag="HL")
                                nc.vector.tensor_scalar(
                                    hl[:], ps[:], 0.0, None, mybir.AluOpType.max
                                )
                                h0_tiles[k] = hl

                # ------------- phase 2: two message-passing rounds ----------
                cur_tiles = h0_tiles
                rnd2_korder = list(range(KT))
                for rnd in (1, 2):
                    with (
                        tc.tile_pool(name=f"rd{rnd}", bufs=1) as rd,
                        tc.tile_pool(name=f"prd{rnd}", bufs=1, space="PSUM") as prd,
                    ):
                        # both dst halves accumulate in ONE [128, 512] psum:
                        # half nh at partitions nh*64, via tile_position
                        # column-groups -- the two M=64 matmuls of each
                        # k-slot run CONCURRENTLY on the PE array
                        psaP = prd.tile([128, 512], F32, tag="psaP")
                        aggP = rd.tile([128, 512], F16, tag="aggP", bufs=2)
                        if rnd == 1:
                            absorb(psaP, 128, 512)
                            hT16 = rd.tile([H, NB], F16, tag="hT16r1")
                            nrm = rd.tile([128, 8 * H], F16, tag="nrm")

                        ks = list(range(KT)) if rnd == 1 else rnd2_korder
                        for ki, k in enumerate(ks):
                            for nh in (0, 1):
                                nc.tensor.matmul(
                                    psaP[nh * H : (nh + 1) * H, :],
                                    cur_tiles[k],
                                    a_slice(k, nh),
                                    start=(ki == 0),
                                    stop=(ki == KT - 1),
                                    tile_position=(0, nh * H),
                                    skip_group_check=True,
                                )

                        # tail: PSUM evacuation split across vector+scalar;
                        # per dst half: W matmul, activation, input DMA,
                        # collective trigger -- each AllGather half fires as
                        # early as possible.
                        nc.vector.tensor_copy(aggP[:, 0:256], psaP[:, 0:256])
                        nc.scalar.copy(aggP[:, 256:512], psaP[:, 256:512])
                        for nh in (0, 1):
                            hsl = slice(nh * H, (nh + 1) * H)
                            nsl = slice(nh * 512, (nh + 1) * 512)
                            psw = prd.tile([H, 512], F32, tag="psw", bufs=2)
                            if nh == 0 and rnd == 1:
                                absorb(psw, H, 512)
                            nc.tensor.matmul(
                                psw[:],
                                wc_s[hsl, :],
                                aggP[hsl, :],
                                start=True,
                                stop=True,
                                tile_position=(nh * H, 0),
                            )
                            hdst = hT16 if rnd == 1 else hT16d[0:H, :]
                            if nh == 0:
                                nc.scalar.activation(
                                    hdst[:, nsl], psw[:], RELU, bias=bc_s[:]
                                )
                            else:
                                nc.vector.tensor_scalar(
                                    hdst[:, nsl],
                                    psw[:],
                                    bc_s[:],
                                    0.0,
                                    mybir.AluOpType.add,
                                    mybir.AluOpType.max,
                                )
                            if rnd == 1:
                                # transpose this half's 4 m-tiles to normal
                                # layout (they are also round-2's own-slot
                                # stationaries), one DMA, one trigger
                                for mm in range(MT // 2):
                                    m = nh * (MT // 2) + mm
                                    pst = prd.tile([128, H], F16, tag="pst", bufs=2)
                                    nc.tensor.transpose(
                                        pst[:],
                                        hT16[:, m * 128 : (m + 1) * 128],
                                        ident[:],
                                    )
                                    if mm % 2 == 0:
                                        nc.vector.tensor_copy(
                                            nrm[:, m * H : (m + 1) * H], pst[:]
                                        )
                                    else:
                                        nc.scalar.copy(
                                            nrm[:, m * H : (m + 1) * H], pst[:]
                                        )
                                agi, ago = (
                                    (ag1a_in, ag1a_out) if nh == 0
                                    else (ag1b_in, ag1b_out)
                                )
                                eng = nc.sync if nh == 0 else nc.scalar
                                eng.dma_start(
                                    out=agi[:].rearrange("(t p) c -> p t c", p=128),
                                    in_=nrm[
                                        :, nh * 4 * H : (nh + 1) * 4 * H
                                    ].rearrange("p (t c) -> p t c", t=4),
                                )
                            else:
                                agi, ago = (
                                    (ag2a_in, ag2a_out) if nh == 0
                                    else (ag2b_in, ag2b_out)
                                )
                                eng = nc.sync if nh == 0 else nc.scalar
                                eng.dma_start(out=agi[:], in_=hT16d[0:H, nsl])
                            nc.gpsimd.collective_compute(
                                "AllGather",
                                mybir.AluOpType.bypass,
                                replica_groups=rg,
                                ins=[agi[:]],
                                outs=[ago[:]],
                            )

                        if rnd == 1:
                            # round-2 stationaries come from the gathered
                            # halves in rank order (static reads; the
                            # per-queue dynamic-DMA register file is
                            # reserved for the phase-3 strip gathers)
                            cur_tiles = [None] * KT
                            korder = []
                            for half, ago in ((0, ag1a_out), (1, ag1b_out)):
                                for r in range(8):
                                    hl8 = hpool.tile(
                                        [128, 4 * H], F16,
                                        name=f"h1_{half}_{r}", tag="HL8", bufs=16,
                                    )
                                    eng = nc.sync if r % 2 == 0 else nc.scalar
                                    eng.dma_start(
                                        out=hl8[:].rearrange(
                                            "p (t c) -> p t c", t=4
                                        ),
                                        in_=ago[
                                            r * 512 : (r + 1) * 512, :
                                        ].rearrange("(t p) c -> p t c", p=128),
                                    )
                                    for t in range(4):
                                        k = 8 * r + 4 * half + t
                                        cur_tiles[k] = hl8[:, t * H : (t + 1) * H]
                                        korder.append(k)
                            rnd2_korder = korder
                        else:
                            # duplicate final h to partitions 64:128 for the
                            # tile_position-paired matmuls
                            nc.scalar.dma_start(
                                out=hT16d[H:128, :], in_=hT16d[0:H, :]
                            )

            # ---------------- phase 3: sim upper cells + output -------------
            # 18 [512x512] cells as even/odd tile_position pairs; stationary
            # = own h strip (hT16d), moving = rotated strips in rhs2.
            with (
                tc.tile_pool(name="ph3", bufs=1) as ph3,
                tc.tile_pool(name="stg", bufs=1) as stg,
                tc.tile_pool(name="pp3", bufs=8, space="PSUM") as pp3,
            ):
                rhs2 = ph3.tile([128, 14 * 512], F16, tag="rhs2")

                def rbase(rho):
                    return H * ((rho // 2) % 2)

                def rcol(rho):
                    return (rho // 2) - 1 if rho % 2 == 0 else 6 + rho // 2

                def issue_gathers(rhos, eng):
                    # rotated strip reads in first-needed order.  Evens
                    # (wait AG2a) ride the sync queue, whose store stream
                    # is idle exactly then (own stores done, even stores
                    # not ready); odds (wait AG2b) ride scalar, free once
                    # the even-cell copies drain.  Keeping gather gens off
                    # the copy-issuing sequencer is what lets PSUM recycle:
                    # a gather gen (~1us with its DynSlice MULTIPLYs)
                    # blocks every copy queued behind it.
                    for rho in rhos:
                        v = rot_vals[rho // 2 - 1]
                        src = ag2a_out if rho % 2 == 0 else ag2b_out
                        eng.dma_start(
                            out=rhs2[
                                rbase(rho) : rbase(rho) + H,
                                rcol(rho) * 512 : (rcol(rho) + 1) * 512,
                            ],
                            in_=src[DynSlice(v * H, H), :],
                        )

                def mov(rho):
                    # moving operand of cell rho; own strips from hT16d
                    if rho == 0:
                        return hT16d[0:H, 0:512]
                    if rho == 1:
                        return hT16d[H:128, 512:1024]
                    b = rbase(rho)
                    return rhs2[b : b + H, rcol(rho) * 512 : (rcol(rho) + 1) * 512]

                first = True
                ncopy = 0
                for phase in ("own", "even", "odd"):
                    if phase == "even":
                        issue_gathers(EVEN_RHO_ORDER, nc.sync)
                        issue_gathers(ODD_RHO_ORDER, nc.scalar)
                    for tau in range(8):
                        sigma, mt = tau // 4, tau % 4
                        chunk = slice(
                            sigma * 512 + mt * 128, sigma * 512 + (mt + 1) * 128
                        )
                        slot0, nsl = PHASE_SLOTS[(sigma, phase)]
                        stA = stg.tile(
                            [128, 4 * 512], BF16, tag=f"st_{phase}", bufs=4
                        )
                        # diagonal cells ((sigma,0) / (sigma,1) vs own strip
                        # sigma) only need their upper triangle: columns
                        # mt*128 and up of this tau's row chunk
                        dlo = mt * 128
                        for rho0, rho64 in SCHED[sigma][phase]:
                            for rho, pbase in ((rho0, 0), (rho64, H)):
                                if rho is None:
                                    continue
                                diag = phase == "own" and rho == sigma
                                clo = dlo if diag else 0
                                ps3 = pp3.tile([128, 512], F32, tag="ps3", bufs=8)
                                if first:
                                    absorb(ps3, 128, 512)
                                    first = False
                                nc.tensor.matmul(
                                    ps3[:],
                                    hT16d[pbase : pbase + H, chunk],
                                    mov(rho),
                                    start=True,
                                    stop=True,
                                    tile_position=(pbase, 0),
                                    skip_group_check=True,
                                )
                                slot = OUT_SLOT[(sigma, rho)] - slot0
                                dst = stA[
                                    :, slot * 512 + clo : (slot + 1) * 512
                                ]
                                # PSUM evacuation is the cell-rate limiter
                                # (~820ns per [128,512] copy, gpsimd can't
                                # read PSUM): alternate scalar/vector
                                if ncopy % 2 == 0:
                                    nc.scalar.copy(dst, ps3[:, clo:512])
                                else:
                                    nc.vector.tensor_copy(dst, ps3[:, clo:512])
                                ncopy += 1
                        rsl = slice(tau * 128, (tau + 1) * 128)
                        # own phase: the diagonal cell is always the first
                        # slot, so start the store at its triangle edge.
                        # odd phase: split the store in two so the drain
                        # starts after the first pair's copies land.
                        olo = dlo if phase == "own" else 0
                        if phase == "odd":
                            h1 = 2 * 512
                            nc.sync.dma_start(
                                out=out_ext[rsl, slot0 * 512 : slot0 * 512 + h1],
                                in_=stA[:, 0:h1],
                            )
                            nc.sync.dma_start(
                                out=out_ext[
                                    rsl, slot0 * 512 + h1 : (slot0 + nsl) * 512
                                ],
                                in_=stA[:, h1 : nsl * 512],
                            )
                        else:
                            nc.sync.dma_start(
                                out=out_ext[
                                    rsl, slot0 * 512 + olo : (slot0 + nsl) * 512
                                ],
                                in_=stA[:, olo : nsl * 512],
                            )
    _legalize_waits(nc)
    return nc


def _host_prep(features, W_node, b_node, W_conv, b_conv, nodes, edges):
    features = np.asarray(features, np.float32)
    W_node = np.asarray(W_node, np.float32)
    b_node = np.asarray(b_node, np.float32)
    W_conv = np.asarray(W_conv, np.float32)
    b_conv = np.asarray(b_conv, np.float32)
    edges = np.asarray(edges)

    def _hilo(x):
        hi = x.astype(ml_dtypes.bfloat16)
        lo = (x - hi.astype(np.float32)).astype(ml_dtypes.bfloat16)
        return hi, lo

    # [features.T; ones] and [W_node; b_node], K-stacked for bf16 hi/lo:
    # [fa_hi; fa_lo_z; fa_hi] . [Wa_hi; Wa_hi; Wa_lo] ~= f@W + b
    fa = np.concatenate([features.T, np.ones((1, N), np.float32)], axis=0)
    Wa = np.concatenate([W_node, b_node[None, :]], axis=0)
    fa_hi, fa_lo = _hilo(fa)
    fa_lo_z = fa_lo.copy()
    fa_lo_z[F, :] = 0  # no double-counted bias
    Wa_hi, Wa_lo = _hilo(Wa)
    featT3 = np.concatenate([fa_hi, fa_lo_z, fa_hi], axis=0)  # [33, N] bf16
    W3 = np.concatenate([Wa_hi, Wa_hi, Wa_lo], axis=0)  # [33, H] bf16
    ftk = featT3.reshape(3 * FA, KT, 128)

    # split into even / odd 128-col k-chunks (see _build_nc phase 1)
    featEv = np.ascontiguousarray(ftk[:, 0::2, :].reshape(3 * FA, N // 2))
    featOd = np.ascontiguousarray(ftk[:, 1::2, :].reshape(3 * FA, N // 2))

    src = edges[:, 0].astype(np.int64)
    dst = edges[:, 1].astype(np.int64)
    in_maps = []
    for c in range(NCORES):
        sel = (dst >= c * NB) & (dst < (c + 1) * NB)
        idx = src[sel] * NB + (dst[sel] - c * NB)
        cnt = np.bincount(idx, minlength=N * NB).astype(np.float32).reshape(N, NB)
        cnt[c * NB + np.arange(NB), np.arange(NB)] += 1.0  # fold identity
        assert cnt.max() <= 16, "adjacency counts exceed exact fp8 range"
        A_pm = np.ascontiguousarray(
            cnt.reshape(KT, 128, NB).transpose(1, 0, 2).reshape(128, KT * NB)
        ).astype(ml_dtypes.float8_e4m3)
        in_maps.append(
            {
                "featEv": featEv,
                "featOd": featOd,
                "W3": W3,
                "Wc16": W_conv.astype(np.float16),
                "bc": b_conv.reshape(H, 1),
                "rot_idx": np.asarray(
                    [(c + k) % 8 for k in range(1, 8)], np.int32
                )[None, :],
                "A_p": A_pm,
            }
        )
    return in_maps


def _assemble(results, nodes):
    """Scatter per-core sim cells into [2, N, N] fp32; mirror and mask."""
    out = np.empty((2, N, N), np.float32)
    sim = out[1]
    for c in range(NCORES):
        T = rot_table(c)
        o = np.asarray(results[c]["out"]).astype(np.float32)  # [1024, 5120]
        for (sigma, rho), slot in OUT_SLOT.items():
            i, j = 2 * c + sigma, T[rho]
            B = o[sigma * 512 : (sigma + 1) * 512, slot * 512 : (slot + 1) * 512]
            if i == j:
                # the device ships only the upper triangle of diagonal cells
                B = np.triu(B) + np.triu(B, 1).T
            sim[i * 512 : (i + 1) * 512, j * 512 : (j + 1) * 512] = B
            if i != j:
                sim[j * 512 : (j + 1) * 512, i * 512 : (i + 1) * 512] = B.T
    m = (np.asarray(nodes) == 2).astype(np.float32)
    np.multiply(sim, m[:, None], out=out[0])
    np.multiply(out[0], m[None, :], out=out[0])
    return out


def kernel(features, W_node, b_node, W_conv, b_conv, nodes, edges, **kw):
    global LAST_RESULT
    _ensure_trace_hook()
    in_maps = _host_prep(features, W_node, b_node, W_conv, b_conv, nodes, edges)
    nc = _build_nc()
    res = run_bass_kernel_spmd(nc, in_maps, core_ids=list(range(NCORES)))
    LAST_RESULT = res
    return _assemble(res.results, nodes)


if __name__ == "__main__":
    np.random.seed(0)
    feats = np.random.randn(N, F).astype(np.float32)
    ins = {
        "features": feats,
        "W_node": (np.random.randn(F, H) * 0.1).astype(np.float32),
        "b_node": (np.random.randn(H) * 0.1).astype(np.float32),
        "W_conv": (np.random.randn(H, H) * 0.05).astype(np.float32),
        "b_conv": (np.random.randn(H) * 0.05).astype(np.float32),
        "nodes": np.random.randint(0, 5, N, dtype=np.int32),
        "edges": np.random.randint(0, N, (524288, 2), dtype=np.int32),
    }
    out = kernel(**ins)
    print(out.shape, out.dtype)


# revision 46
# speedup vs baseline: 1.0121x; 1.0121x over previous
"""Trainium2 Bass kernel for the DependencyAnalyzer GNN problem.

Computation (reference semantics):
    h = relu(features @ W_node + b_node)                  # [N, H]
    2x: agg = scatter_add(h[src] -> dst);  h = relu((h + agg) @ W_conv + b_conv)
    out = stack([ (m*h) @ (m*h).T,  h @ h.T ])            # m = (nodes == 2)

Strategy (8 NeuronCores, SPMD):
  - Host reformats the edge list into per-core dense adjacency blocks
    A'^T [src=8192, dst_local=1024] in fp8 (counts are exact), with the
    identity folded in (A' = A + I_c) so that A' @ h == h_block + agg.
    The src k-tiles are PERMUTED per core: own block first, then peers
    in ring order (c+1, ..., c+7), with features permuted identically,
    so round 2 starts on locally-available own tiles before AG1 lands
    and consumes each peer's tiles in gather-arrival order -- while the
    instruction stream stays core-uniform (peer addressing goes through
    DynSlice registers loaded from a per-core index input).
  - h is fp16 end-to-end (validated: 3.6e-3 max rel err vs the 2e-2
    gate).  Each round ends in TWO AllGather halves; measured mesh time
    is bytes-dominated (~70 GB/s/core + ~5us fixed), and the collective
    engine has a ~55-70us cold-init wall after kernel launch, so the
    split halves pipeline compute into the second mesh: round 2 runs
    during AG1b, the first sim cells during AG2a/b.
  - Both outputs are symmetric and function_deps = mask.outer * sim, so
    the device computes ONLY the upper triangle of sim: a uniform
    18-cell-per-core cover of the 136 upper [512x512] cells.  Cells run
    as tile_position row-group pairs (~2x over serial K=64 matmuls).
  - sim cells are written as bf16; the host casts, mirrors, and applies
    the fdeps mask during output assembly.
"""

import numpy as np
import ml_dtypes

import concourse.bass as bass
import concourse.mybir as mybir
import concourse.tile as tile
from concourse import masks
from concourse.bass import DynSlice
from concourse.bass_utils import run_bass_kernel_spmd

N = 8192          # nodes
NB = 1024         # nodes per core block
NCORES = 8
F = 10            # feature dim
FA = F + 1        # +1 ones row (bias fold)
H = 64            # hidden dim
KT = N // 128     # 64 src k-tiles
MT = NB // 128    # 8 own m-tiles
F32 = mybir.dt.float32
F16 = mybir.dt.float16
BF16 = mybir.dt.bfloat16
F8 = mybir.dt.float8e4
I32 = mybir.dt.int32
RELU = mybir.ActivationFunctionType.Relu

# ---- the 18-cell symmetric cover -----------------------------------------
# cell = (sigma, rho): sim[own strip sigma (512 rows)] x [rot strip rho],
# rot strip rho = absolute strip (2c + rho) % 16 (pure rotation).  rho 0,1
# are the core's own strips.  Cell (1, 8) is dropped everywhere: its pair
# {2c+1, 2c+8} is exactly core (c+4)'s (0, 9) pair, so the 19-cell
# rotation cover is uniformly redundant there.  Cells run as tile_position
# row-group pairs (rho@rows0:64, rho'@64:128).  Gathered strip rho sits at
# partition base 64*((rho//2)%2), column slot (rho//2)-1 for evens /
# 6+rho//2 for odds of the rhs tile.  Schedule per sigma: "own" runs
# before the final AllGathers, "even" after AG2a, "odd" after AG2b.
SCHED = {
    0: {"own": [(0, 1)], "even": [(4, 2), (8, 6)], "odd": [(9, 11), (13, 15)]},
    1: {"own": [(None, 1)], "even": [(12, 10), (None, 14)], "odd": [(5, 3), (9, 7)]},
}
# output column slot (x512) in out_ext for each (sigma, rho) cell
OUT_SLOT = {
    (0, 0): 0, (0, 1): 1, (0, 4): 2, (0, 2): 3, (0, 8): 4, (0, 6): 5,
    (0, 9): 6, (0, 11): 7, (0, 13): 8, (0, 15): 9,
    (1, 1): 0, (1, 12): 1, (1, 10): 2, (1, 14): 3,
    (1, 5): 4, (1, 3): 5, (1, 9): 6, (1, 7): 7,
}
# first slot and slot count of each (sigma, phase) output store
PHASE_SLOTS = {
    (0, "own"): (0, 2), (0, "even"): (2, 4), (0, "odd"): (6, 4),
    (1, "own"): (0, 1), (1, "even"): (1, 3), (1, "odd"): (4, 4),
}
# rotated-strip gather issue order = first-needed order in the tau loop
EVEN_RHO_ORDER = [4, 2, 8, 6, 12, 10, 14]
ODD_RHO_ORDER = [9, 11, 13, 15, 5, 3, 7]


def rot_table(c):
    """Absolute 512-strip index for each rotated slot rho of core c."""
    return [(2 * c + r) % 16 for r in range(16)]


def k_perm(c):
    """Per-core src k-tile permutation: perm[slot] = absolute k-tile.
    Own block (8 tiles) first, then peer (c+j)'s first-half tiles for
    j=1..7 (delivered by AG1a), then the peers' second halves (AG1b)."""
    perm = [8 * c + t for t in range(8)]
    perm += [8 * ((c + j) % 8) + t for j in range(1, 8) for t in range(4)]
    perm += [8 * ((c + j) % 8) + 4 + t for j in range(1, 8) for t in range(4)]
    return perm


LAST_RESULT = None  # BassKernelResults of the most recent run (for test harness)


def _ensure_trace_hook():
    """Best-effort: register the NTFF profiling hook for trace=True runs."""
    import sys as _sys
    import types as _types

    try:
        if "antenv.axon_hooks" in _sys.modules:
            return
        import antenv as _antenv

        mod = _types.ModuleType("antenv.axon_hooks")
        _state = {"hook": None}
        mod.set_axon_ntff_profile_hook = lambda h: _state.__setitem__("hook", h)
        mod.get_axon_ntff_profile_hook = lambda: _state["hook"]
        _sys.modules["antenv.axon_hooks"] = mod
        _antenv.axon_hooks = mod

        from trn_agent_boot.trn_boot import _ntff_profile_via_ctypes

        so_path = "/opt/axon/libaxon_pjrt.so"
        import os as _os

        if _os.path.exists(so_path):
            hook = _ntff_profile_via_ctypes(so_path)
            if hook is not None:
                mod.set_axon_ntff_profile_hook(hook)
    except Exception:
        pass


def _legalize_waits(nc, max_waits=1):
    """This walrus build accepts at most one sync-wait per lowered HW
    instruction; hoist extra waits onto standalone EventSemaphore
    instructions on the same (in-order) engine queue."""
    n_fixed = 0
    for f in nc.m.functions:
        for bb in f.blocks:
            new_list = []
            for ins in bb.instructions:
                si = ins.sync_info
                if si is not None and len(si.on_wait) > max_waits:
                    waits = list(si.on_wait)
                    for w in waits[: len(waits) - max_waits]:
                        ev = mybir.InstEventSemaphore(
                            name=f"{ins.name}-w-{w.ant_name}",
                            ins=[],
                            outs=[],
                            sync_info=mybir.SyncInfo(on_wait=[w], on_update=[]),
                            engine=ins.engine,
                        )
                        new_list.append(ev)
                    ins.sync_info = mybir.SyncInfo(
                        on_wait=waits[len(waits) - max_waits :],
                        on_update=list(si.on_update),
                    )
                    n_fixed += 1
                new_list.append(ins)
            bb.instructions = new_list
    return n_fixed


def _build_nc():
    nc = bass.Bass(num_devices=NCORES)

    # ---- external I/O (same program on all cores; per-core data differs) ----
    # features^T, k-slots permuted per core (see k_perm), split by slot
    # parity: even slots at SBUF partitions 0:33, odd at 64:97 for
    # concurrent 2-row-tile phase-1 matmul pairs.
    featEv = nc.declare_dram_parameter("featEv", [3 * FA, N // 2], BF16, isOutput=False)
    featOd = nc.declare_dram_parameter("featOd", [3 * FA, N // 2], BF16, isOutput=False)
    WnA = nc.declare_dram_parameter("W3", [3 * FA, H], BF16, isOutput=False)
    Wc16 = nc.declare_dram_parameter("Wc16", [H, H], F16, isOutput=False)
    bc = nc.declare_dram_parameter("bc", [H, 1], F32, isOutput=False)
    rot_idx = nc.declare_dram_parameter("rot_idx", [1, 7], I32, isOutput=False)
    # A'^T p-major, k-slots permuted: A_p[p, s*1024 + n] = A'^T[perm[s]*128+p, n]
    A_p = nc.declare_dram_parameter("A_p", [128, KT * NB], F8, isOutput=False)
    # out[tau*128+p, slot*512 + f]: sim cell values (see OUT_SLOT)
    out_ext = nc.declare_dram_parameter("out", [NB, 10 * 512], BF16, isOutput=True)

    # ---- internal DRAM (collective bounce buffers) ----
    ag1a_in = nc.dram_tensor("ag1a_in", [NB // 2, H], F16)
    ag1a_out = nc.dram_tensor("ag1a_out", [N // 2, H], F16, addr_space="Shared")
    ag1b_in = nc.dram_tensor("ag1b_in", [NB // 2, H], F16)
    ag1b_out = nc.dram_tensor("ag1b_out", [N // 2, H], F16, addr_space="Shared")
    # final h, fp16, T layout: AG2a carries every core's even strip (local
    # cols 0:512), AG2b the odd strip; out row r*64+k = rank r's row k
    ag2a_in = nc.dram_tensor("ag2a_in", [H, 512], F16)
    ag2a_out = nc.dram_tensor("ag2a_out", [8 * H, 512], F16, addr_space="Shared")
    ag2b_in = nc.dram_tensor("ag2b_in", [H, 512], F16)
    ag2b_out = nc.dram_tensor("ag2b_out", [8 * H, 512], F16, addr_space="Shared")
    rg = [list(range(NCORES))]

    with tile.TileContext(nc, num_cores=NCORES) as tc:
        with tc.tile_pool(name="persist", bufs=1) as persist:
            # ---------------- constants / small inputs ----------------------
            # sync queue: W then features (phase-1 critical path) then half
            # the A tiles; scalar queue: small constants + other A half.
            wn_s = persist.tile([64 + 3 * FA, H], BF16)
            nc.sync.dma_start(out=wn_s[0 : 3 * FA, :], in_=WnA[:])
            nc.sync.dma_start(out=wn_s[64 : 64 + 3 * FA, :], in_=WnA[:])
            # W_conv on both partition halves so the two dst-half W matmuls
            # can run as a tile_position row-group pair
            wc_s = persist.tile([128, H], F16)
            nc.scalar.dma_start(out=wc_s[0:H, :], in_=Wc16[:])
            nc.scalar.dma_start(out=wc_s[H:128, :], in_=Wc16[:])
            bc_s = persist.tile([H, 1], F32)
            nc.scalar.dma_start(out=bc_s[:], in_=bc[:])
            rot_s = persist.tile([1, 7], I32)
            nc.scalar.dma_start(out=rot_s[:], in_=rot_idx[:])
            ident = persist.tile([H, H], F16)
            masks.make_identity(nc, ident[:])
            dummy_s = persist.tile([1, 512], BF16)
            nc.vector.memset(dummy_s[:], 0.0)

            # ring indices (c+j)%8, j=1..7 -> registers for the per-core
            # peer reads out of the final AllGathers
            rot_vals = [
                nc.values_load(
                    rot_s[0:1, i : i + 1],
                    min_val=0,
                    max_val=7,
                    skip_runtime_bounds_check=True,
                )
                for i in range(7)
            ]
            # scratch for the register-load warm-up (see phase 1)
            rwarm = persist.tile([1, 16], F8)

            def absorb(pt, parts, free):
                # Dummy full-tile matmul: soaks up PSUM pool-boundary WAR
                # waits on PE so real matmuls stay within the ISA's sync
                # wait budget.
                nc.tensor.matmul(
                    pt[:, 0:free],
                    dummy_s[0:1, 0:parts],
                    dummy_s[0:1, 0:free],
                    start=True,
                    stop=True,
                )

            # final h (own block, T layout, fp16), duplicated on partitions
            # 64:128 for tile_position-paired K=64 matmuls in phase 3
            hT16d = persist.tile([128, NB], F16)

            with (
                tc.tile_pool(name="apool", bufs=16) as apool,
                tc.tile_pool(name="hpool", bufs=KT) as hpool,
            ):
                # ------------- phase 1: h0 for all nodes (replicated) -------
                # Concurrent row-tile pairs: even k-slot at partitions 0:33
                # (tile (0,0)), odd at 64:97 (tile (64,0)).  PSUM is
                # evacuated on the vector engine: the scalar sequencer is
                # busy generating A-tile DMA descriptors at this point.
                h0_tiles = [None] * KT
                with (
                    tc.tile_pool(name="ph1", bufs=2) as ph1,
                    tc.tile_pool(name="pp1", bufs=4, space="PSUM") as pp1,
                ):
                    ft_halves = []
                    for half in range(2):
                        ft_h = ph1.tile(
                            [64 + 3 * FA, N // 4], BF16, tag=f"ft{half}", bufs=1
                        )
                        nc.sync.dma_start(
                            out=ft_h[0 : 3 * FA, :],
                            in_=featEv[:, half * (N // 4) : (half + 1) * (N // 4)],
                        )
                        nc.sync.dma_start(
                            out=ft_h[64 : 64 + 3 * FA, :],
                            in_=featOd[:, half * (N // 4) : (half + 1) * (N // 4)],
                        )
                        ft_halves.append(ft_h)

                    # adjacency, fp8, resident in SBUF for both rounds;
                    # alternate queues so descriptor gen is 2-wide
                    a_tiles = []
                    for j in range(16):
                        at = apool.tile([128, 4 * NB], F8, name=f"a{j}", tag="A")
                        eng = nc.sync if j % 2 == 0 else nc.scalar
                        eng.dma_start(
                            out=at[:], in_=A_p[:, j * 4 * NB : (j + 1) * 4 * NB]
                        )
                        a_tiles.append(at)

                    def a_slice(k, nh):
                        t = a_tiles[k // 4]
                        off = (k % 4) * NB + nh * 512
                        return t[:, off : off + 512]

                    # The per-queue DynSlice register loads otherwise
                    # materialize lazily at first use (~0.35us each, right
                    # in the AG2->cells gap).  Touch each register now on
                    # the queues that will run the phase-3 strip gathers
                    # (scalar: even strips, gpsimd: odd strips) -- 1-byte
                    # dynamic reads of the read-only adjacency input,
                    # emitted behind the A-tile descriptor gens so the A
                    # load isn't delayed.
                    for i, v in enumerate(rot_vals):
                        nc.scalar.dma_start(
                            out=rwarm[0:1, i : i + 1], in_=A_p[0:1, DynSlice(v, 1)]
                        )
                        nc.gpsimd.dma_start(
                            out=rwarm[0:1, i + 8 : i + 9],
                            in_=A_p[0:1, DynSlice(v, 1)],
                        )

                    first_p1 = True
                    for half in range(2):
                        ft_h = ft_halves[half]
                        for j in range(KT // 4):  # 16 pairs per half
                            csl = slice(j * 128, (j + 1) * 128)
                            for par, pbase in ((0, 0), (1, 64)):
                                k = half * (KT // 2) + 2 * j + par
                                ps = pp1.tile([128, H], F32, tag="p64", bufs=4)
                                if first_p1:
                                    absorb(ps, 128, H)
                                    first_p1 = False
                                nc.tensor.matmul(
                                    ps[:],
                                    ft_h[pbase : pbase + 3 * FA, csl],
                                    wn_s[pbase : pbase + 3 * FA, :],
                                    start=True,
                                    stop=True,
                                    tile_position=(pbase, 0),
                                    skip_group_check=True,
                                )
                                hl = hpool.tile([128, H], F16, name=f"h0_{k}", tag="HL")
                                nc.vector.tensor_scalar(
                                    hl[:], ps[:], 0.0, None, mybir.AluOpType.max
                                )
                                h0_tiles[k] = hl

                # ------------- phase 2: two message-passing rounds ----------
                cur_tiles = h0_tiles
                rnd2_korder = list(range(KT))
                for rnd in (1, 2):
                    with (
                        tc.tile_pool(name=f"rd{rnd}", bufs=1) as rd,
                        tc.tile_pool(name=f"prd{rnd}", bufs=1, space="PSUM") as prd,
                    ):
                        # both dst halves accumulate in ONE [128, 512] psum:
                        # half nh at partitions nh*64, via tile_position
                        # column-groups -- the two M=64 matmuls of each
                        # k-slot run CONCURRENTLY on the PE array
                        psaP = prd.tile([128, 512], F32, tag="psaP")
                        aggP = rd.tile([128, 512], F16, tag="aggP", bufs=2)
                        if rnd == 1:
                            absorb(psaP, 128, 512)
                            hT16 = rd.tile([H, NB], F16, tag="hT16r1")
                            nrm = rd.tile([128, 8 * H], F16, tag="nrm")

                        ks = list(range(KT)) if rnd == 1 else rnd2_korder
                        for ki, k in enumerate(ks):
                            for nh in (0, 1):
                                nc.tensor.matmul(
                                    psaP[nh * H : (nh + 1) * H, :],
                                    cur_tiles[k],
                                    a_slice(k, nh),
                                    start=(ki == 0),
                                    stop=(ki == KT - 1),
                                    tile_position=(0, nh * H),
                                    skip_group_check=True,
                                )

                        # tail: PSUM evacuation split across vector+scalar;
                        # per dst half: W matmul, activation, input DMA,
                        # collective trigger -- each AllGather half fires as
                        # early as possible.
                        nc.vector.tensor_copy(aggP[:, 0:256], psaP[:, 0:256])
                        nc.scalar.copy(aggP[:, 256:512], psaP[:, 256:512])
                        for nh in (0, 1):
                            hsl = slice(nh * H, (nh + 1) * H)
                            nsl = slice(nh * 512, (nh + 1) * 512)
                            psw = prd.tile([H, 512], F32, tag="psw", bufs=2)
                            if nh == 0 and rnd == 1:
                                absorb(psw, H, 512)
                            nc.tensor.matmul(
                                psw[:],
                                wc_s[hsl, :],
                                aggP[hsl, :],
                                start=True,
                                stop=True,
                                tile_position=(nh * H, 0),
                            )
                            hdst = hT16 if rnd == 1 else hT16d[0:H, :]
                            if nh == 0:
                                nc.scalar.activation(
                                    hdst[:, nsl], psw[:], RELU, bias=bc_s[:]
                                )
                            else:
                                nc.vector.tensor_scalar(
                                    hdst[:, nsl],
                                    psw[:],
                                    bc_s[:],
                                    0.0,
                                    mybir.AluOpType.add,
                                    mybir.AluOpType.max,
                                )
                            if rnd == 1:
                                # transpose this half's 4 m-tiles to normal
                                # layout (they are also round-2's own-slot
                                # stationaries), one DMA, one trigger
                                for mm in range(MT // 2):
                                    m = nh * (MT // 2) + mm
                                    pst = prd.tile([128, H], F16, tag="pst", bufs=2)
                                    nc.tensor.transpose(
                                        pst[:],
                                        hT16[:, m * 128 : (m + 1) * 128],
                                        ident[:],
                                    )
                                    if mm % 2 == 0:
                                        nc.vector.tensor_copy(
                                            nrm[:, m * H : (m + 1) * H], pst[:]
                                        )
                                    else:
                                        nc.scalar.copy(
                                            nrm[:, m * H : (m + 1) * H], pst[:]
                                        )
                                agi, ago = (
                                    (ag1a_in, ag1a_out) if nh == 0
                                    else (ag1b_in, ag1b_out)
                                )
                                eng = nc.sync if nh == 0 else nc.scalar
                                eng.dma_start(
                                    out=agi[:].rearrange("(t p) c -> p t c", p=128),
                                    in_=nrm[
                                        :, nh * 4 * H : (nh + 1) * 4 * H
                                    ].rearrange("p (t c) -> p t c", t=4),
                                )
                            else:
                                agi, ago = (
                                    (ag2a_in, ag2a_out) if nh == 0
                                    else (ag2b_in, ag2b_out)
                                )
                                eng = nc.sync if nh == 0 else nc.scalar
                                eng.dma_start(out=agi[:], in_=hT16d[0:H, nsl])
                            nc.gpsimd.collective_compute(
                                "AllGather",
                                mybir.AluOpType.bypass,
                                replica_groups=rg,
                                ins=[agi[:]],
                                outs=[ago[:]],
                            )

                        if rnd == 1:
                            # round-2 stationaries come from the gathered
                            # halves in rank order (static reads; the
                            # per-queue dynamic-DMA register file is
                            # reserved for the phase-3 strip gathers)
                            cur_tiles = [None] * KT
                            korder = []
                            for half, ago in ((0, ag1a_out), (1, ag1b_out)):
                                for r in range(8):
                                    hl8 = hpool.tile(
                                        [128, 4 * H], F16,
                                        name=f"h1_{half}_{r}", tag="HL8", bufs=16,
                                    )
                                    eng = nc.sync if r % 2 == 0 else nc.scalar
                                    eng.dma_start(
                                        out=hl8[:].rearrange(
                                            "p (t c) -> p t c", t=4
                                        ),
                                        in_=ago[
                                            r * 512 : (r + 1) * 512, :
                                        ].rearrange("(t p) c -> p t c", p=128),
                                    )
                                    for t in range(4):
                                        k = 8 * r + 4 * half + t
                                        cur_tiles[k] = hl8[:, t * H : (t + 1) * H]
                                        korder.append(k)
                            rnd2_korder = korder
                        else:
                            # duplicate final h to partitions 64:128 for the
                            # tile_position-paired matmuls
                            nc.scalar.dma_start(
                                out=hT16d[H:128, :], in_=hT16d[0:H, :]
                            )

            # ---------------- phase 3: sim upper cells + output -------------
            # 18 [512x512] cells as even/odd tile_position pairs; stationary
            # = own h strip (hT16d), moving = rotated strips in rhs2.
            with (
                tc.tile_pool(name="ph3", bufs=1) as ph3,
                tc.tile_pool(name="stg", bufs=1) as stg,
                tc.tile_pool(name="pp3", bufs=8, space="PSUM") as pp3,
            ):
                rhs2 = ph3.tile([128, 14 * 512], F16, tag="rhs2")

                def rbase(rho):
                    return H * ((rho // 2) % 2)

                def rcol(rho):
                    return (rho // 2) - 1 if rho % 2 == 0 else 6 + rho // 2

                def issue_gathers(rhos, eng):
                    # rotated strip reads in first-needed order.  Evens
                    # (wait AG2a) ride the scalar queue; odds (wait AG2b)
                    # ride the idle gpsimd software-DGE queue -- on scalar
                    # the tile scheduler parks them behind the even-cell
                    # PSUM copies, landing the odd strips ~6us late.  The
                    # sync queue is reserved for output stores throughout.
                    for rho in rhos:
                        v = rot_vals[rho // 2 - 1]
                        src = ag2a_out if rho % 2 == 0 else ag2b_out
                        eng.dma_start(
                            out=rhs2[
                                rbase(rho) : rbase(rho) + H,
                                rcol(rho) * 512 : (rcol(rho) + 1) * 512,
                            ],
                            in_=src[DynSlice(v * H, H), :],
                        )

                def mov(rho):
                    # moving operand of cell rho; own strips from hT16d
                    if rho == 0:
                        return hT16d[0:H, 0:512]
                    if rho == 1:
                        return hT16d[H:128, 512:1024]
                    b = rbase(rho)
                    return rhs2[b : b + H, rcol(rho) * 512 : (rcol(rho) + 1) * 512]

                first = True
                ncopy = 0
                for phase in ("own", "even", "odd"):
                    if phase == "even":
                        issue_gathers(EVEN_RHO_ORDER, nc.scalar)
                        issue_gathers(ODD_RHO_ORDER, nc.gpsimd)
                    for tau in range(8):
                        sigma, mt = tau // 4, tau % 4
                        chunk = slice(
                            sigma * 512 + mt * 128, sigma * 512 + (mt + 1) * 128
                        )
                        slot0, nsl = PHASE_SLOTS[(sigma, phase)]
                        stA = stg.tile(
                            [128, 4 * 512], BF16, tag=f"st_{phase}", bufs=4
                        )
                        # diagonal cells ((sigma,0) / (sigma,1) vs own strip
                        # sigma) only need their upper triangle: columns
                        # mt*128 and up of this tau's row chunk
                        dlo = mt * 128
                        for rho0, rho64 in SCHED[sigma][phase]:
                            pr = [
                                (r, pb)
                                for r, pb in ((rho0, 0), (rho64, H))
                                if r is not None
                            ]
                            wide = len(pr)
                            # A pair's two cells land in ADJACENT output
                            # slots, so the pair accumulates into one
                            # 2-bank PSUM tile and evacuates with a single
                            # double-width copy -- PSUM evacuation is the
                            # cell-rate limiter and per-op overhead is a
                            # big slice of each copy (72 -> 40 copies).
                            if wide == 2:
                                psP = pp3.tile(
                                    [128, 1024], F32, tag="ps3w", bufs=3,
                                    name="psP",
                                )
                            else:
                                psP = pp3.tile(
                                    [128, 512], F32, tag="ps3s", bufs=2,
                                    name="psS",
                                )
                            if first:
                                absorb(psP, 128, 512)
                                first = False
                            clo = (
                                dlo
                                if (phase == "own" and pr[0][0] == sigma)
                                else 0
                            )
                            for idx, (rho, pbase) in enumerate(pr):
                                nc.tensor.matmul(
                                    psP[:, idx * 512 : (idx + 1) * 512],
                                    hT16d[pbase : pbase + H, chunk],
                                    mov(rho),
                                    start=True,
                                    stop=True,
                                    tile_position=(pbase, 0),
                                    skip_group_check=True,
                                )
                            slotA = OUT_SLOT[(sigma, pr[0][0])] - slot0
                            dst = stA[
                                :, slotA * 512 + clo : (slotA + wide) * 512
                            ]
                            if ncopy % 2 == 0:
                                nc.scalar.copy(dst, psP[:, clo : wide * 512])
                            else:
                                nc.vector.tensor_copy(
                                    dst, psP[:, clo : wide * 512]
                                )
                            ncopy += 1
                        rsl = slice(tau * 128, (tau + 1) * 128)
                        # own phase: the diagonal cell is always the first
                        # slot, so start the store at its triangle edge.
                        # odd phase: split the store in two so the drain
                        # starts after the first pair's copies land.
                        olo = dlo if phase == "own" else 0
                        if phase == "odd":
                            h1 = 2 * 512
                            nc.sync.dma_start(
                                out=out_ext[rsl, slot0 * 512 : slot0 * 512 + h1],
                                in_=stA[:, 0:h1],
                            )
                            nc.sync.dma_start(
                                out=out_ext[
                                    rsl, slot0 * 512 + h1 : (slot0 + nsl) * 512
                                ],
                                in_=stA[:, h1 : nsl * 512],
                            )
                        else:
                            nc.sync.dma_start(
                                out=out_ext[
                                    rsl, slot0 * 512 + olo : (slot0 + nsl) * 512
                                ],
                                in_=stA[:, olo : nsl * 512],
                            )
    _legalize_waits(nc)
    return nc


def _host_prep(features, W_node, b_node, W_conv, b_conv, nodes, edges):
    features = np.asarray(features, np.float32)
    W_node = np.asarray(W_node, np.float32)
    b_node = np.asarray(b_node, np.float32)
    W_conv = np.asarray(W_conv, np.float32)
    b_conv = np.asarray(b_conv, np.float32)
    edges = np.asarray(edges)

    def _hilo(x):
        hi = x.astype(ml_dtypes.bfloat16)
        lo = (x - hi.astype(np.float32)).astype(ml_dtypes.bfloat16)
        return hi, lo

    # [features.T; ones] and [W_node; b_node], K-stacked for bf16 hi/lo:
    # [fa_hi; fa_lo_z; fa_hi] . [Wa_hi; Wa_hi; Wa_lo] ~= f@W + b
    fa = np.concatenate([features.T, np.ones((1, N), np.float32)], axis=0)
    Wa = np.concatenate([W_node, b_node[None, :]], axis=0)
    fa_hi, fa_lo = _hilo(fa)
    fa_lo_z = fa_lo.copy()
    fa_lo_z[F, :] = 0  # no double-counted bias
    Wa_hi, Wa_lo = _hilo(Wa)
    featT3 = np.concatenate([fa_hi, fa_lo_z, fa_hi], axis=0)  # [33, N] bf16
    W3 = np.concatenate([Wa_hi, Wa_hi, Wa_lo], axis=0)  # [33, H] bf16
    ftk = featT3.reshape(3 * FA, KT, 128)

    # split into even / odd 128-col k-chunks (see _build_nc phase 1)
    featEv = np.ascontiguousarray(ftk[:, 0::2, :].reshape(3 * FA, N // 2))
    featOd = np.ascontiguousarray(ftk[:, 1::2, :].reshape(3 * FA, N // 2))

    src = edges[:, 0].astype(np.int64)
    dst = edges[:, 1].astype(np.int64)
    in_maps = []
    for c in range(NCORES):
        sel = (dst >= c * NB) & (dst < (c + 1) * NB)
        idx = src[sel] * NB + (dst[sel] - c * NB)
        cnt = np.bincount(idx, minlength=N * NB).astype(np.float32).reshape(N, NB)
        cnt[c * NB + np.arange(NB), np.arange(NB)] += 1.0  # fold identity
        assert cnt.max() <= 16, "adjacency counts exceed exact fp8 range"
        A_pm = np.ascontiguousarray(
            cnt.reshape(KT, 128, NB).transpose(1, 0, 2).reshape(128, KT * NB)
        ).astype(ml_dtypes.float8_e4m3)
        in_maps.append(
            {
                "featEv": featEv,
                "featOd": featOd,
                "W3": W3,
                "Wc16": W_conv.astype(np.float16),
                "bc": b_conv.reshape(H, 1),
                "rot_idx": np.asarray(
                    [(c + k) % 8 for k in range(1, 8)], np.int32
                )[None, :],
                "A_p": A_pm,
            }
        )
    return in_maps


def _assemble(results, nodes):
    """Scatter per-core sim cells into [2, N, N] fp32; mirror and mask."""
    out = np.empty((2, N, N), np.float32)
    sim = out[1]
    for c in range(NCORES):
        T = rot_table(c)
        o = np.asarray(results[c]["out"]).astype(np.float32)  # [1024, 5120]
        for (sigma, rho), slot in OUT_SLOT.items():
            i, j = 2 * c + sigma, T[rho]
            B = o[sigma * 512 : (sigma + 1) * 512, slot * 512 : (slot + 1) * 512]
            if i == j:
                # the device ships only the upper triangle of diagonal cells
                B = np.triu(B) + np.triu(B, 1).T
            sim[i * 512 : (i + 1) * 512, j * 512 : (j + 1) * 512] = B
            if i != j:
                sim[j * 512 : (j + 1) * 512, i * 512 : (i + 1) * 512] = B.T
    m = (np.asarray(nodes) == 2).astype(np.float32)
    np.multiply(sim, m[:, None], out=out[0])
    np.multiply(out[0], m[None, :], out=out[0])
    return out


def kernel(features, W_node, b_node, W_conv, b_conv, nodes, edges, **kw):
    global LAST_RESULT
    _ensure_trace_hook()
    in_maps = _host_prep(features, W_node, b_node, W_conv, b_conv, nodes, edges)
    nc = _build_nc()
    res = run_bass_kernel_spmd(nc, in_maps, core_ids=list(range(NCORES)))
    LAST_RESULT = res
    return _assemble(res.results, nodes)


if __name__ == "__main__":
    np.random.seed(0)
    feats = np.random.randn(N, F).astype(np.float32)
    ins = {
        "features": feats,
        "W_node": (np.random.randn(F, H) * 0.1).astype(np.float32),
        "b_node": (np.random.randn(H) * 0.1).astype(np.float32),
        "W_conv": (np.random.randn(H, H) * 0.05).astype(np.float32),
        "b_conv": (np.random.randn(H) * 0.05).astype(np.float32),
        "nodes": np.random.randint(0, 5, N, dtype=np.int32),
        "edges": np.random.randint(0, N, (524288, 2), dtype=np.int32),
    }
    out = kernel(**ins)
    print(out.shape, out.dtype)


# revision 49
# speedup vs baseline: 1.0559x; 1.0432x over previous
"""Trainium2 Bass kernel for the DependencyAnalyzer GNN problem.

Computation (reference semantics):
    h = relu(features @ W_node + b_node)                  # [N, H]
    2x: agg = scatter_add(h[src] -> dst);  h = relu((h + agg) @ W_conv + b_conv)
    out = stack([ (m*h) @ (m*h).T,  h @ h.T ])            # m = (nodes == 2)

Strategy (8 NeuronCores, SPMD):
  - Host reformats the edge list into per-core dense adjacency blocks
    A'^T [src=8192, dst_local=1024] in fp8 (counts are exact), with the
    identity folded in (A' = A + I_c) so that A' @ h == h_block + agg.
    The src k-tiles are PERMUTED per core: own block first, then peers
    in ring order (c+1, ..., c+7), with features permuted identically,
    so round 2 starts on locally-available own tiles before AG1 lands
    and consumes each peer's tiles in gather-arrival order -- while the
    instruction stream stays core-uniform (peer addressing goes through
    DynSlice registers loaded from a per-core index input).
  - h is fp16 end-to-end (validated: 3.6e-3 max rel err vs the 2e-2
    gate).  Each round ends in TWO AllGather halves; measured mesh time
    is bytes-dominated (~70 GB/s/core + ~5us fixed), and the collective
    engine has a ~55-70us cold-init wall after kernel launch, so the
    split halves pipeline compute into the second mesh: round 2 runs
    during AG1b, the first sim cells during AG2a/b.
  - Both outputs are symmetric and function_deps = mask.outer * sim, so
    the device computes ONLY the upper triangle of sim: a uniform
    18-cell-per-core cover of the 136 upper [512x512] cells.  Cells run
    as tile_position row-group pairs (~2x over serial K=64 matmuls).
  - sim cells are written as bf16; the host casts, mirrors, and applies
    the fdeps mask during output assembly.
"""

import numpy as np
import ml_dtypes

import concourse.bass as bass
import concourse.mybir as mybir
import concourse.tile as tile
from concourse import masks
from concourse.bass import DynSlice
from concourse.bass_utils import run_bass_kernel_spmd

N = 8192          # nodes
NB = 1024         # nodes per core block
NCORES = 8
F = 10            # feature dim
FA = F + 1        # +1 ones row (bias fold)
H = 64            # hidden dim
KT = N // 128     # 64 src k-tiles
MT = NB // 128    # 8 own m-tiles
F32 = mybir.dt.float32
F16 = mybir.dt.float16
BF16 = mybir.dt.bfloat16
F8 = mybir.dt.float8e4
I32 = mybir.dt.int32
RELU = mybir.ActivationFunctionType.Relu

# ---- the 18-cell symmetric cover -----------------------------------------
# cell = (sigma, rho): sim[own strip sigma (512 rows)] x [rot strip rho],
# rot strip rho = absolute strip (2c + rho) % 16 (pure rotation).  rho 0,1
# are the core's own strips.  Cell (1, 8) is dropped everywhere: its pair
# {2c+1, 2c+8} is exactly core (c+4)'s (0, 9) pair, so the 19-cell
# rotation cover is uniformly redundant there.  Cells run as tile_position
# row-group pairs (rho@rows0:64, rho'@64:128).  Gathered strip rho sits at
# partition base 64*((rho//2)%2), column slot (rho//2)-1 for evens /
# 6+rho//2 for odds of the rhs tile.  Schedule per sigma: "own" runs
# before the final AllGathers, "even" after AG2a, "odd" after AG2b.
SCHED = {
    0: {"own": [(0, 1)], "even": [(4, 2), (8, 6)], "odd": [(9, 11), (13, 15)]},
    1: {"own": [(None, 1)], "even": [(12, 10), (None, 14)], "odd": [(5, 3), (9, 7)]},
}
# output column slot (x512) in out_ext for each (sigma, rho) cell
OUT_SLOT = {
    (0, 0): 0, (0, 1): 1, (0, 4): 2, (0, 2): 3, (0, 8): 4, (0, 6): 5,
    (0, 9): 6, (0, 11): 7, (0, 13): 8, (0, 15): 9,
    (1, 1): 0, (1, 12): 1, (1, 10): 2, (1, 14): 3,
    (1, 5): 4, (1, 3): 5, (1, 9): 6, (1, 7): 7,
}
# first slot and slot count of each (sigma, phase) output store
PHASE_SLOTS = {
    (0, "own"): (0, 2), (0, "even"): (2, 4), (0, "odd"): (6, 4),
    (1, "own"): (0, 1), (1, "even"): (1, 3), (1, "odd"): (4, 4),
}
# rotated-strip gather issue order = first-needed order in the tau loop
EVEN_RHO_ORDER = [4, 2, 8, 6, 12, 10, 14]
ODD_RHO_ORDER = [9, 11, 13, 15, 5, 3, 7]


def rot_table(c):
    """Absolute 512-strip index for each rotated slot rho of core c."""
    return [(2 * c + r) % 16 for r in range(16)]


def k_perm(c):
    """Per-core src k-tile permutation: perm[slot] = absolute k-tile.
    Own block (8 tiles) first, then peer (c+j)'s first-half tiles for
    j=1..7 (delivered by AG1a), then the peers' second halves (AG1b)."""
    perm = [8 * c + t for t in range(8)]
    perm += [8 * ((c + j) % 8) + t for j in range(1, 8) for t in range(4)]
    perm += [8 * ((c + j) % 8) + 4 + t for j in range(1, 8) for t in range(4)]
    return perm


LAST_RESULT = None  # BassKernelResults of the most recent run (for test harness)


def _ensure_trace_hook():
    """Best-effort: register the NTFF profiling hook for trace=True runs."""
    import sys as _sys
    import types as _types

    try:
        if "antenv.axon_hooks" in _sys.modules:
            return
        import antenv as _antenv

        mod = _types.ModuleType("antenv.axon_hooks")
        _state = {"hook": None}
        mod.set_axon_ntff_profile_hook = lambda h: _state.__setitem__("hook", h)
        mod.get_axon_ntff_profile_hook = lambda: _state["hook"]
        _sys.modules["antenv.axon_hooks"] = mod
        _antenv.axon_hooks = mod

        from trn_agent_boot.trn_boot import _ntff_profile_via_ctypes

        so_path = "/opt/axon/libaxon_pjrt.so"
        import os as _os

        if _os.path.exists(so_path):
            hook = _ntff_profile_via_ctypes(so_path)
            if hook is not None:
                mod.set_axon_ntff_profile_hook(hook)
    except Exception:
        pass


def _legalize_waits(nc, max_waits=1):
    """This walrus build accepts at most one sync-wait per lowered HW
    instruction; hoist extra waits onto standalone EventSemaphore
    instructions on the same (in-order) engine queue."""
    n_fixed = 0
    for f in nc.m.functions:
        for bb in f.blocks:
            new_list = []
            for ins in bb.instructions:
                si = ins.sync_info
                if si is not None and len(si.on_wait) > max_waits:
                    waits = list(si.on_wait)
                    for w in waits[: len(waits) - max_waits]:
                        ev = mybir.InstEventSemaphore(
                            name=f"{ins.name}-w-{w.ant_name}",
                            ins=[],
                            outs=[],
                            sync_info=mybir.SyncInfo(on_wait=[w], on_update=[]),
                            engine=ins.engine,
                        )
                        new_list.append(ev)
                    ins.sync_info = mybir.SyncInfo(
                        on_wait=waits[len(waits) - max_waits :],
                        on_update=list(si.on_update),
                    )
                    n_fixed += 1
                new_list.append(ins)
            bb.instructions = new_list
    return n_fixed


def _build_nc():
    nc = bass.Bass(num_devices=NCORES)

    # ---- external I/O (same program on all cores; per-core data differs) ----
    # features^T, k-slots permuted per core (see k_perm), split by slot
    # parity: even slots at SBUF partitions 0:33, odd at 64:97 for
    # concurrent 2-row-tile phase-1 matmul pairs.
    featEv = nc.declare_dram_parameter("featEv", [3 * FA, N // 2], BF16, isOutput=False)
    featOd = nc.declare_dram_parameter("featOd", [3 * FA, N // 2], BF16, isOutput=False)
    WnA = nc.declare_dram_parameter("W3", [3 * FA, H], BF16, isOutput=False)
    Wc16 = nc.declare_dram_parameter("Wc16", [H, H], F16, isOutput=False)
    bc = nc.declare_dram_parameter("bc", [H, 1], F32, isOutput=False)
    rot_idx = nc.declare_dram_parameter("rot_idx", [1, 7], I32, isOutput=False)
    # A'^T p-major, k-slots permuted: A_p[p, s*1024 + n] = A'^T[perm[s]*128+p, n]
    A_p = nc.declare_dram_parameter("A_p", [128, KT * NB], F8, isOutput=False)
    # out[tau*128+p, slot*512 + f]: sim cell values (see OUT_SLOT)
    out_ext = nc.declare_dram_parameter("out", [NB, 10 * 512], BF16, isOutput=True)

    # ---- internal DRAM (collective bounce buffers) ----
    ag1a_in = nc.dram_tensor("ag1a_in", [NB // 2, H], F16)
    ag1a_out = nc.dram_tensor("ag1a_out", [N // 2, H], F16, addr_space="Shared")
    ag1b_in = nc.dram_tensor("ag1b_in", [NB // 2, H], F16)
    ag1b_out = nc.dram_tensor("ag1b_out", [N // 2, H], F16, addr_space="Shared")
    # final h, fp16, T layout: AG2a carries every core's even strip (local
    # cols 0:512), AG2b the odd strip; out row r*64+k = rank r's row k
    ag2a_in = nc.dram_tensor("ag2a_in", [H, 512], F16)
    ag2a_out = nc.dram_tensor("ag2a_out", [8 * H, 512], F16, addr_space="Shared")
    ag2b_in = nc.dram_tensor("ag2b_in", [H, 512], F16)
    ag2b_out = nc.dram_tensor("ag2b_out", [8 * H, 512], F16, addr_space="Shared")
    rg = [list(range(NCORES))]

    with tile.TileContext(nc, num_cores=NCORES) as tc:
        with tc.tile_pool(name="persist", bufs=1) as persist:
            # ---------------- constants / small inputs ----------------------
            # sync queue: W then features (phase-1 critical path) then half
            # the A tiles; scalar queue: small constants + other A half.
            wn_s = persist.tile([64 + 3 * FA, H], BF16)
            nc.sync.dma_start(out=wn_s[0 : 3 * FA, :], in_=WnA[:])
            nc.sync.dma_start(out=wn_s[64 : 64 + 3 * FA, :], in_=WnA[:])
            # W_conv on both partition halves so the two dst-half W matmuls
            # can run as a tile_position row-group pair
            wc_s = persist.tile([128, H], F16)
            nc.scalar.dma_start(out=wc_s[0:H, :], in_=Wc16[:])
            nc.scalar.dma_start(out=wc_s[H:128, :], in_=Wc16[:])
            bc_s = persist.tile([H, 1], F32)
            nc.scalar.dma_start(out=bc_s[:], in_=bc[:])
            rot_s = persist.tile([1, 7], I32)
            nc.scalar.dma_start(out=rot_s[:], in_=rot_idx[:])
            ident = persist.tile([H, H], F16)
            masks.make_identity(nc, ident[:])
            dummy_s = persist.tile([1, 512], BF16)
            nc.vector.memset(dummy_s[:], 0.0)

            # ring indices (c+j)%8, j=1..7 -> registers for the per-core
            # peer reads out of the final AllGathers
            rot_vals = [
                nc.values_load(
                    rot_s[0:1, i : i + 1],
                    min_val=0,
                    max_val=7,
                    skip_runtime_bounds_check=True,
                )
                for i in range(7)
            ]
            # scratch for the register-load warm-up (see phase 1)
            rwarm = persist.tile([1, 16], F8)

            def absorb(pt, parts, free):
                # Dummy full-tile matmul: soaks up PSUM pool-boundary WAR
                # waits on PE so real matmuls stay within the ISA's sync
                # wait budget.
                nc.tensor.matmul(
                    pt[:, 0:free],
                    dummy_s[0:1, 0:parts],
                    dummy_s[0:1, 0:free],
                    start=True,
                    stop=True,
                )

            # final h (own block, T layout, fp16), duplicated on partitions
            # 64:128 for tile_position-paired K=64 matmuls in phase 3
            hT16d = persist.tile([128, NB], F16)

            with (
                tc.tile_pool(name="apool", bufs=16) as apool,
                tc.tile_pool(name="hpool", bufs=KT) as hpool,
            ):
                # ------------- phase 1: h0 for all nodes (replicated) -------
                # Concurrent row-tile pairs: even k-slot at partitions 0:33
                # (tile (0,0)), odd at 64:97 (tile (64,0)).  PSUM is
                # evacuated on the vector engine: the scalar sequencer is
                # busy generating A-tile DMA descriptors at this point.
                h0_tiles = [None] * KT
                with (
                    tc.tile_pool(name="ph1", bufs=2) as ph1,
                    tc.tile_pool(name="pp1", bufs=4, space="PSUM") as pp1,
                ):
                    ft_halves = []
                    for half in range(2):
                        ft_h = ph1.tile(
                            [64 + 3 * FA, N // 4], BF16, tag=f"ft{half}", bufs=1
                        )
                        nc.sync.dma_start(
                            out=ft_h[0 : 3 * FA, :],
                            in_=featEv[:, half * (N // 4) : (half + 1) * (N // 4)],
                        )
                        nc.sync.dma_start(
                            out=ft_h[64 : 64 + 3 * FA, :],
                            in_=featOd[:, half * (N // 4) : (half + 1) * (N // 4)],
                        )
                        ft_halves.append(ft_h)

                    # adjacency, fp8, resident in SBUF for both rounds;
                    # alternate queues so descriptor gen is 2-wide
                    a_tiles = []
                    for j in range(16):
                        at = apool.tile([128, 4 * NB], F8, name=f"a{j}", tag="A")
                        eng = nc.sync if j % 2 == 0 else nc.scalar
                        eng.dma_start(
                            out=at[:], in_=A_p[:, j * 4 * NB : (j + 1) * 4 * NB]
                        )
                        a_tiles.append(at)

                    def a_slice(k, nh):
                        t = a_tiles[k // 4]
                        off = (k % 4) * NB + nh * 512
                        return t[:, off : off + 512]

                    # The per-queue DynSlice register loads otherwise
                    # materialize lazily at first use (~0.35us each, right
                    # in the AG2->cells gap).  Touch each register now on
                    # the queues that will run the phase-3 strip gathers
                    # (scalar: even strips, gpsimd: odd strips) -- 1-byte
                    # dynamic reads of the read-only adjacency input,
                    # emitted behind the A-tile descriptor gens so the A
                    # load isn't delayed.
                    for i, v in enumerate(rot_vals):
                        nc.scalar.dma_start(
                            out=rwarm[0:1, i : i + 1], in_=A_p[0:1, DynSlice(v, 1)]
                        )
                        nc.gpsimd.dma_start(
                            out=rwarm[0:1, i + 8 : i + 9],
                            in_=A_p[0:1, DynSlice(v, 1)],
                        )

                    first_p1 = True
                    for half in range(2):
                        ft_h = ft_halves[half]
                        for j in range(KT // 4):  # 16 pairs per half
                            csl = slice(j * 128, (j + 1) * 128)
                            for par, pbase in ((0, 0), (1, 64)):
                                k = half * (KT // 2) + 2 * j + par
                                ps = pp1.tile([128, H], F32, tag="p64", bufs=4)
                                if first_p1:
                                    absorb(ps, 128, H)
                                    first_p1 = False
                                nc.tensor.matmul(
                                    ps[:],
                                    ft_h[pbase : pbase + 3 * FA, csl],
                                    wn_s[pbase : pbase + 3 * FA, :],
                                    start=True,
                                    stop=True,
                                    tile_position=(pbase, 0),
                                    skip_group_check=True,
                                )
                                hl = hpool.tile([128, H], F16, name=f"h0_{k}", tag="HL")
                                nc.vector.tensor_scalar(
                                    hl[:], ps[:], 0.0, None, mybir.AluOpType.max
                                )
                                h0_tiles[k] = hl

                # ------------- phase 2: two message-passing rounds ----------
                cur_tiles = h0_tiles
                rnd2_korder = list(range(KT))
                for rnd in (1, 2):
                    with (
                        tc.tile_pool(name=f"rd{rnd}", bufs=1) as rd,
                        tc.tile_pool(name=f"prd{rnd}", bufs=1, space="PSUM") as prd,
                    ):
                        # both dst halves accumulate in ONE [128, 512] psum:
                        # half nh at partitions nh*64, via tile_position
                        # column-groups -- the two M=64 matmuls of each
                        # k-slot run CONCURRENTLY on the PE array
                        psaP = prd.tile([128, 512], F32, tag="psaP")
                        aggP = rd.tile([128, 512], F16, tag="aggP", bufs=2)
                        if rnd == 1:
                            absorb(psaP, 128, 512)
                            hT16 = rd.tile([H, NB], F16, tag="hT16r1")
                            nrm = rd.tile([128, 8 * H], F16, tag="nrm")

                        # Round 1 / round-2 batch 1 (the AG1a k-slots):
                        # (k, dst-half) column-group pairs into psaP.
                        ks = list(range(KT)) if rnd == 1 else rnd2_korder[:32]
                        for ki, k in enumerate(ks):
                            for nh in (0, 1):
                                nc.tensor.matmul(
                                    psaP[nh * H : (nh + 1) * H, :],
                                    cur_tiles[k],
                                    a_slice(k, nh),
                                    start=(ki == 0),
                                    stop=(ki == len(ks) - 1),
                                    tile_position=(0, nh * H),
                                    skip_group_check=True,
                                )

                        if rnd == 2:
                            # Batch 2 (AG1b k-slots) runs dst-half0 FIRST
                            # with k-parity column pairing, so AG2a fires
                            # ~3.4us before round 2 fully ends and its mesh
                            # overlaps the dst-half1 matmuls.  Column group
                            # par takes k-slot b2[2j+par]; the two partial
                            # sums land at PSUM partitions 0:64 / 64:128
                            # and the K=128 [W;W] conv matmul re-sums them
                            # together with the batch-1 partials.
                            b2 = rnd2_korder[32:]
                            # stage the batch-1 partials to SBUF now: the
                            # copies overlap the batch-2 matmuls, and the
                            # tail's merge add may read only ONE PSUM
                            # operand (HW restriction)
                            aggb0 = rd.tile([128, 512], F16, tag="aggb0")
                            aggb1 = rd.tile([128, 512], F16, tag="aggb1")
                            aggb = [aggb0, aggb1]
                            nc.vector.tensor_copy(aggb0[0:H, :], psaP[0:H, :])
                            nc.scalar.copy(aggb1[H:128, :], psaP[H:128, :])
                            for nh in (0, 1):
                                psX = prd.tile(
                                    [128, 512], F32, tag=f"psb{nh}",
                                    name=f"psb{nh}",
                                )
                                for j in range(16):
                                    for par in (0, 1):
                                        k = b2[2 * j + par]
                                        nc.tensor.matmul(
                                            psX[par * H : (par + 1) * H, :],
                                            cur_tiles[k],
                                            a_slice(k, nh),
                                            start=(j == 0),
                                            stop=(j == 15),
                                            tile_position=(0, par * H),
                                            skip_group_check=True,
                                        )
                                aggX = aggb[nh]
                                # merge the aligned partials on vector
                                # (SBUF in-place + one PSUM read), copy the
                                # remaining bank half on scalar
                                msl = slice(nh * H, (nh + 1) * H)
                                osl = slice((1 - nh) * H, (2 - nh) * H)
                                nc.vector.scalar_tensor_tensor(
                                    aggX[msl, :],
                                    aggX[msl, :],
                                    0.0,
                                    psX[msl, :],
                                    mybir.AluOpType.add,
                                    mybir.AluOpType.add,
                                )
                                nc.scalar.copy(aggX[osl, :], psX[osl, :])
                                psw = prd.tile([H, 512], F32, tag="psw", bufs=2)
                                nc.tensor.matmul(
                                    psw[:],
                                    wc_s[:, :],
                                    aggX[:],
                                    start=True,
                                    stop=True,
                                    tile_position=(0, 0),
                                )
                                nsl = slice(nh * 512, (nh + 1) * 512)
                                if nh == 0:
                                    nc.scalar.activation(
                                        hT16d[0:H, nsl], psw[:], RELU,
                                        bias=bc_s[:],
                                    )
                                else:
                                    nc.vector.tensor_scalar(
                                        hT16d[0:H, nsl],
                                        psw[:],
                                        bc_s[:],
                                        0.0,
                                        mybir.AluOpType.add,
                                        mybir.AluOpType.max,
                                    )
                                agi, ago = (
                                    (ag2a_in, ag2a_out) if nh == 0
                                    else (ag2b_in, ag2b_out)
                                )
                                nc.sync.dma_start(out=agi[:], in_=hT16d[0:H, nsl])
                                nc.gpsimd.collective_compute(
                                    "AllGather",
                                    mybir.AluOpType.bypass,
                                    replica_groups=rg,
                                    ins=[agi[:]],
                                    outs=[ago[:]],
                                )
                            # duplicate final h to partitions 64:128 for the
                            # tile_position-paired phase-3 matmuls
                            nc.scalar.dma_start(
                                out=hT16d[H:128, :], in_=hT16d[0:H, :]
                            )
                            continue

                        # tail: PSUM evacuation split across vector+scalar;
                        # per dst half: W matmul, activation, input DMA,
                        # collective trigger -- each AllGather half fires as
                        # early as possible.
                        nc.vector.tensor_copy(aggP[:, 0:256], psaP[:, 0:256])
                        nc.scalar.copy(aggP[:, 256:512], psaP[:, 256:512])
                        for nh in (0, 1):
                            hsl = slice(nh * H, (nh + 1) * H)
                            nsl = slice(nh * 512, (nh + 1) * 512)
                            psw = prd.tile([H, 512], F32, tag="psw", bufs=2)
                            if nh == 0 and rnd == 1:
                                absorb(psw, H, 512)
                            nc.tensor.matmul(
                                psw[:],
                                wc_s[hsl, :],
                                aggP[hsl, :],
                                start=True,
                                stop=True,
                                tile_position=(nh * H, 0),
                            )
                            hdst = hT16 if rnd == 1 else hT16d[0:H, :]
                            if nh == 0:
                                nc.scalar.activation(
                                    hdst[:, nsl], psw[:], RELU, bias=bc_s[:]
                                )
                            else:
                                nc.vector.tensor_scalar(
                                    hdst[:, nsl],
                                    psw[:],
                                    bc_s[:],
                                    0.0,
                                    mybir.AluOpType.add,
                                    mybir.AluOpType.max,
                                )
                            if rnd == 1:
                                # transpose this half's 4 m-tiles to normal
                                # layout (they are also round-2's own-slot
                                # stationaries), one DMA, one trigger
                                for mm in range(MT // 2):
                                    m = nh * (MT // 2) + mm
                                    pst = prd.tile([128, H], F16, tag="pst", bufs=2)
                                    nc.tensor.transpose(
                                        pst[:],
                                        hT16[:, m * 128 : (m + 1) * 128],
                                        ident[:],
                                    )
                                    if mm % 2 == 0:
                                        nc.vector.tensor_copy(
                                            nrm[:, m * H : (m + 1) * H], pst[:]
                                        )
                                    else:
                                        nc.scalar.copy(
                                            nrm[:, m * H : (m + 1) * H], pst[:]
                                        )
                                agi, ago = (
                                    (ag1a_in, ag1a_out) if nh == 0
                                    else (ag1b_in, ag1b_out)
                                )
                                eng = nc.sync if nh == 0 else nc.scalar
                                eng.dma_start(
                                    out=agi[:].rearrange("(t p) c -> p t c", p=128),
                                    in_=nrm[
                                        :, nh * 4 * H : (nh + 1) * 4 * H
                                    ].rearrange("p (t c) -> p t c", t=4),
                                )
                            else:
                                agi, ago = (
                                    (ag2a_in, ag2a_out) if nh == 0
                                    else (ag2b_in, ag2b_out)
                                )
                                eng = nc.sync if nh == 0 else nc.scalar
                                eng.dma_start(out=agi[:], in_=hT16d[0:H, nsl])
                            nc.gpsimd.collective_compute(
                                "AllGather",
                                mybir.AluOpType.bypass,
                                replica_groups=rg,
                                ins=[agi[:]],
                                outs=[ago[:]],
                            )

                        if rnd == 1:
                            # round-2 stationaries come from the gathered
                            # halves in rank order (static reads; the
                            # per-queue dynamic-DMA register file is
                            # reserved for the phase-3 strip gathers)
                            cur_tiles = [None] * KT
                            korder = []
                            for half, ago in ((0, ag1a_out), (1, ag1b_out)):
                                for r in range(8):
                                    hl8 = hpool.tile(
                                        [128, 4 * H], F16,
                                        name=f"h1_{half}_{r}", tag="HL8", bufs=16,
                                    )
                                    eng = nc.sync if r % 2 == 0 else nc.scalar
                                    eng.dma_start(
                                        out=hl8[:].rearrange(
                                            "p (t c) -> p t c", t=4
                                        ),
                                        in_=ago[
                                            r * 512 : (r + 1) * 512, :
                                        ].rearrange("(t p) c -> p t c", p=128),
                                    )
                                    for t in range(4):
                                        k = 8 * r + 4 * half + t
                                        cur_tiles[k] = hl8[:, t * H : (t + 1) * H]
                                        korder.append(k)
                            rnd2_korder = korder
                        else:
                            # duplicate final h to partitions 64:128 for the
                            # tile_position-paired matmuls
                            nc.scalar.dma_start(
                                out=hT16d[H:128, :], in_=hT16d[0:H, :]
                            )

            # ---------------- phase 3: sim upper cells + output -------------
            # 18 [512x512] cells as even/odd tile_position pairs; stationary
            # = own h strip (hT16d), moving = rotated strips in rhs2.
            with (
                tc.tile_pool(name="ph3", bufs=1) as ph3,
                tc.tile_pool(name="stg", bufs=1) as stg,
                tc.tile_pool(name="pp3", bufs=8, space="PSUM") as pp3,
            ):
                rhs2 = ph3.tile([128, 14 * 512], F16, tag="rhs2")

                def rbase(rho):
                    return H * ((rho // 2) % 2)

                def rcol(rho):
                    return (rho // 2) - 1 if rho % 2 == 0 else 6 + rho // 2

                def issue_gathers(rhos, eng):
                    # rotated strip reads in first-needed order.  Evens
                    # (wait AG2a) ride the scalar queue; odds (wait AG2b)
                    # ride the idle gpsimd software-DGE queue -- on scalar
                    # the tile scheduler parks them behind the even-cell
                    # PSUM copies, landing the odd strips ~6us late.  The
                    # sync queue is reserved for output stores throughout.
                    for rho in rhos:
                        v = rot_vals[rho // 2 - 1]
                        src = ag2a_out if rho % 2 == 0 else ag2b_out
                        eng.dma_start(
                            out=rhs2[
                                rbase(rho) : rbase(rho) + H,
                                rcol(rho) * 512 : (rcol(rho) + 1) * 512,
                            ],
                            in_=src[DynSlice(v * H, H), :],
                        )

                def mov(rho):
                    # moving operand of cell rho; own strips from hT16d
                    if rho == 0:
                        return hT16d[0:H, 0:512]
                    if rho == 1:
                        return hT16d[H:128, 512:1024]
                    b = rbase(rho)
                    return rhs2[b : b + H, rcol(rho) * 512 : (rcol(rho) + 1) * 512]

                first = True
                ncopy = 0
                for phase in ("own", "even", "odd"):
                    if phase == "even":
                        issue_gathers(EVEN_RHO_ORDER, nc.scalar)
                        issue_gathers(ODD_RHO_ORDER, nc.gpsimd)
                    for tau in range(8):
                        sigma, mt = tau // 4, tau % 4
                        chunk = slice(
                            sigma * 512 + mt * 128, sigma * 512 + (mt + 1) * 128
                        )
                        slot0, nsl = PHASE_SLOTS[(sigma, phase)]
                        stA = stg.tile(
                            [128, 4 * 512], BF16, tag=f"st_{phase}", bufs=4
                        )
                        # diagonal cells ((sigma,0) / (sigma,1) vs own strip
                        # sigma) only need their upper triangle: columns
                        # mt*128 and up of this tau's row chunk
                        dlo = mt * 128
                        for rho0, rho64 in SCHED[sigma][phase]:
                            pr = [
                                (r, pb)
                                for r, pb in ((rho0, 0), (rho64, H))
                                if r is not None
                            ]
                            wide = len(pr)
                            # A pair's two cells land in ADJACENT output
                            # slots, so the pair accumulates into one
                            # 2-bank PSUM tile and evacuates with a single
                            # double-width copy -- PSUM evacuation is the
                            # cell-rate limiter and per-op overhead is a
                            # big slice of each copy (72 -> 40 copies).
                            if wide == 2:
                                psP = pp3.tile(
                                    [128, 1024], F32, tag="ps3w", bufs=3,
                                    name="psP",
                                )
                            else:
                                psP = pp3.tile(
                                    [128, 512], F32, tag="ps3s", bufs=2,
                                    name="psS",
                                )
                            if first:
                                absorb(psP, 128, 512)
                                first = False
                            clo = (
                                dlo
                                if (phase == "own" and pr[0][0] == sigma)
                                else 0
                            )
                            for idx, (rho, pbase) in enumerate(pr):
                                nc.tensor.matmul(
                                    psP[:, idx * 512 : (idx + 1) * 512],
                                    hT16d[pbase : pbase + H, chunk],
                                    mov(rho),
                                    start=True,
                                    stop=True,
                                    tile_position=(pbase, 0),
                                    skip_group_check=True,
                                )
                            slotA = OUT_SLOT[(sigma, pr[0][0])] - slot0
                            dst = stA[
                                :, slotA * 512 + clo : (slotA + wide) * 512
                            ]
                            if ncopy % 2 == 0:
                                nc.scalar.copy(dst, psP[:, clo : wide * 512])
                            else:
                                nc.vector.tensor_copy(
                                    dst, psP[:, clo : wide * 512]
                                )
                            ncopy += 1
                        rsl = slice(tau * 128, (tau + 1) * 128)
                        # own phase: the diagonal cell is always the first
                        # slot, so start the store at its triangle edge.
                        # odd phase: split the store in two so the drain
                        # starts after the first pair's copies land.
                        olo = dlo if phase == "own" else 0
                        if phase == "odd":
                            h1 = 2 * 512
                            nc.sync.dma_start(
                                out=out_ext[rsl, slot0 * 512 : slot0 * 512 + h1],
                                in_=stA[:, 0:h1],
                            )
                            nc.sync.dma_start(
                                out=out_ext[
                                    rsl, slot0 * 512 + h1 : (slot0 + nsl) * 512
                                ],
                                in_=stA[:, h1 : nsl * 512],
                            )
                        else:
                            nc.sync.dma_start(
                                out=out_ext[
                                    rsl, slot0 * 512 + olo : (slot0 + nsl) * 512
                                ],
                                in_=stA[:, olo : nsl * 512],
                            )
    _legalize_waits(nc)
    return nc


def _host_prep(features, W_node, b_node, W_conv, b_conv, nodes, edges):
    features = np.asarray(features, np.float32)
    W_node = np.asarray(W_node, np.float32)
    b_node = np.asarray(b_node, np.float32)
    W_conv = np.asarray(W_conv, np.float32)
    b_conv = np.asarray(b_conv, np.float32)
    edges = np.asarray(edges)

    def _hilo(x):
        hi = x.astype(ml_dtypes.bfloat16)
        lo = (x - hi.astype(np.float32)).astype(ml_dtypes.bfloat16)
        return hi, lo

    # [features.T; ones] and [W_node; b_node], K-stacked for bf16 hi/lo:
    # [fa_hi; fa_lo_z; fa_hi] . [Wa_hi; Wa_hi; Wa_lo] ~= f@W + b
    fa = np.concatenate([features.T, np.ones((1, N), np.float32)], axis=0)
    Wa = np.concatenate([W_node, b_node[None, :]], axis=0)
    fa_hi, fa_lo = _hilo(fa)
    fa_lo_z = fa_lo.copy()
    fa_lo_z[F, :] = 0  # no double-counted bias
    Wa_hi, Wa_lo = _hilo(Wa)
    featT3 = np.concatenate([fa_hi, fa_lo_z, fa_hi], axis=0)  # [33, N] bf16
    W3 = np.concatenate([Wa_hi, Wa_hi, Wa_lo], axis=0)  # [33, H] bf16
    ftk = featT3.reshape(3 * FA, KT, 128)

    # split into even / odd 128-col k-chunks (see _build_nc phase 1)
    featEv = np.ascontiguousarray(ftk[:, 0::2, :].reshape(3 * FA, N // 2))
    featOd = np.ascontiguousarray(ftk[:, 1::2, :].reshape(3 * FA, N // 2))

    src = edges[:, 0].astype(np.int64)
    dst = edges[:, 1].astype(np.int64)
    in_maps = []
    for c in range(NCORES):
        sel = (dst >= c * NB) & (dst < (c + 1) * NB)
        idx = src[sel] * NB + (dst[sel] - c * NB)
        cnt = np.bincount(idx, minlength=N * NB).astype(np.float32).reshape(N, NB)
        cnt[c * NB + np.arange(NB), np.arange(NB)] += 1.0  # fold identity
        assert cnt.max() <= 16, "adjacency counts exceed exact fp8 range"
        A_pm = np.ascontiguousarray(
            cnt.reshape(KT, 128, NB).transpose(1, 0, 2).reshape(128, KT * NB)
        ).astype(ml_dtypes.float8_e4m3)
        in_maps.append(
            {
                "featEv": featEv,
                "featOd": featOd,
                "W3": W3,
                "Wc16": W_conv.astype(np.float16),
                "bc": b_conv.reshape(H, 1),
                "rot_idx": np.asarray(
                    [(c + k) % 8 for k in range(1, 8)], np.int32
                )[None, :],
                "A_p": A_pm,
            }
        )
    return in_maps


def _assemble(results, nodes):
    """Scatter per-core sim cells into [2, N, N] fp32; mirror and mask."""
    out = np.empty((2, N, N), np.float32)
    sim = out[1]
    for c in range(NCORES):
        T = rot_table(c)
        o = np.asarray(results[c]["out"]).astype(np.float32)  # [1024, 5120]
        for (sigma, rho), slot in OUT_SLOT.items():
            i, j = 2 * c + sigma, T[rho]
            B = o[sigma * 512 : (sigma + 1) * 512, slot * 512 : (slot + 1) * 512]
            if i == j:
                # the device ships only the upper triangle of diagonal cells
                B = np.triu(B) + np.triu(B, 1).T
            sim[i * 512 : (i + 1) * 512, j * 512 : (j + 1) * 512] = B
            if i != j:
                sim[j * 512 : (j + 1) * 512, i * 512 : (i + 1) * 512] = B.T
    m = (np.asarray(nodes) == 2).astype(np.float32)
    np.multiply(sim, m[:, None], out=out[0])
    np.multiply(out[0], m[None, :], out=out[0])
    return out


def kernel(features, W_node, b_node, W_conv, b_conv, nodes, edges, **kw):
    global LAST_RESULT
    _ensure_trace_hook()
    in_maps = _host_prep(features, W_node, b_node, W_conv, b_conv, nodes, edges)
    nc = _build_nc()
    res = run_bass_kernel_spmd(nc, in_maps, core_ids=list(range(NCORES)))
    LAST_RESULT = res
    return _assemble(res.results, nodes)


if __name__ == "__main__":
    np.random.seed(0)
    feats = np.random.randn(N, F).astype(np.float32)
    ins = {
        "features": feats,
        "W_node": (np.random.randn(F, H) * 0.1).astype(np.float32),
        "b_node": (np.random.randn(H) * 0.1).astype(np.float32),
        "W_conv": (np.random.randn(H, H) * 0.05).astype(np.float32),
        "b_conv": (np.random.randn(H) * 0.05).astype(np.float32),
        "nodes": np.random.randint(0, 5, N, dtype=np.int32),
        "edges": np.random.randint(0, N, (524288, 2), dtype=np.int32),
    }
    out = kernel(**ins)
    print(out.shape, out.dtype)
